# revision 1
# baseline (speedup 1.0000x reference)
import sys
sys.path.insert(0, '/opt/trn_rl_repo')
import numpy as np
import ml_dtypes

import concourse.bass as bass
import concourse.mybir as mybir
from concourse.bass_utils import run_bass_kernel_spmd

# Problem: y[b,s,o] = x[b]@W.T + bias + (x[b]@a[idx[b]].T)@b[idx[b]].T
# B=8 batch elements -> data-parallel, one per NeuronCore.
B, S, D, RANK = 8, 2048, 4096, 16
P = 128
KT = D // P          # 32 contraction tiles
NQ = 4               # s-quarters
SQ = S // NQ         # 512
NJ = 8               # o-blocks of 512
OJ = D // NJ         # 512
NT = SQ // P         # 4 s-tiles per quarter
NGROUP = NQ * NJ * NT  # 128 output groups of [128 s, 512 o]

_BF = mybir.dt.bfloat16
_F32 = mybir.dt.float32


def build_nc():
    nc = bass.Bass()
    xt = nc.declare_dram_parameter("xt", [D, S], _BF, isOutput=False)
    wt = nc.declare_dram_parameter("wt", [D, D], _BF, isOutput=False)
    at = nc.declare_dram_parameter("at", [D, RANK], _BF, isOutput=False)
    bt = nc.declare_dram_parameter("bt", [RANK + 1, D], _BF, isOutput=False)
    ones = nc.declare_dram_parameter("ones", [1, S], _BF, isOutput=False)
    y = nc.declare_dram_parameter("y", [S, D], _F32, isOutput=True)

    xt_t = xt.rearrange("(k p) s -> p k s", p=P)
    wt_t = wt.rearrange("(k p) o -> p k o", p=P)
    at_t = at.rearrange("(k p) r -> p k r", p=P)

    with (
        nc.sbuf_tensor([P, 2, KT, SQ], _BF) as x_sb,
        nc.sbuf_tensor([P, 2, KT, OJ], _BF) as w_sb,
        nc.sbuf_tensor([P, KT, RANK], _BF) as at_sb,
        nc.sbuf_tensor([RANK + 1, D], _BF) as bt_sb,
        nc.sbuf_tensor([RANK + 1, S], _BF) as inter_sb,
        nc.sbuf_tensor([P, 4, OJ], _F32) as out_sb,
        nc.psum_tensor([P, 7, OJ], _F32) as psum_y,
        nc.psum_tensor([P, SQ], _F32) as psum_i,
        nc.semaphore("x_sem") as x_sem,
        nc.semaphore("w_sem") as w_sem,
        nc.semaphore("c_sem") as c_sem,
        nc.semaphore("pe_sem") as pe_sem,
        nc.semaphore("pei_sem") as pei_sem,
        nc.semaphore("dve_sem") as dve_sem,
        nc.semaphore("ev_sem") as ev_sem,
        nc.semaphore("st_sem") as st_sem,
        nc.Block() as block,
    ):
        @block.sync
        def _(sync):
            sync.dma_start(at_sb[:], at_t).then_inc(c_sem, 16)
            sync.dma_start(bt_sb[:], bt[:, :]).then_inc(c_sem, 16)
            sync.dma_start(inter_sb[RANK:RANK + 1, :], ones[:, :]).then_inc(c_sem, 16)
            for q in range(NQ):
                if q >= 2:
                    sync.wait_ge(ev_sem, NJ * NT * (q - 1))
                sync.dma_start(
                    x_sb[:, q % 2], xt_t[:, :, q * SQ:(q + 1) * SQ]
                ).then_inc(x_sem, 16)
                for j in range(NJ):
                    wj = q * NJ + j
                    if wj >= 2:
                        sync.wait_ge(ev_sem, NT * (wj - 1))
                    sync.dma_start(
                        w_sb[:, j % 2], wt_t[:, :, j * OJ:(j + 1) * OJ]
                    ).then_inc(w_sem, 16)

        @block.tensor
        def _(tensor):
            tensor.wait_ge(c_sem, 48)
            g = 0
            for q in range(NQ):
                tensor.wait_ge(x_sem, 16 * (q + 1))
                if q > 0:
                    tensor.wait_ge(dve_sem, q)     # psum_i WAR
                for i in range(KT):
                    mm = nc.tensor.matmul(
                        psum_i[0:RANK, :], at_sb[:, i, :], x_sb[:, q % 2, i, :],
                        start=(i == 0), stop=(i == KT - 1),
                    )
                mm.then_inc(pei_sem, 1)
                for j in range(NJ):
                    wj = q * NJ + j
                    tensor.wait_ge(w_sem, 16 * (wj + 1))
                    for t in range(NT):
                        st = q * NT + t
                        if g >= 7:
                            tensor.wait_ge(ev_sem, g - 6)
                        for i in range(KT):
                            nc.tensor.matmul(
                                psum_y[:, g % 7, :],
                                x_sb[:, q % 2, i, t * P:(t + 1) * P],
                                w_sb[:, j % 2, i, :],
                                start=(i == 0), stop=False,
                            )
                        tensor.wait_ge(dve_sem, q + 1)
                        nc.tensor.matmul(
                            psum_y[:, g % 7, :],
                            inter_sb[:, st * P:(st + 1) * P],
                            bt_sb[:, j * OJ:(j + 1) * OJ],
                            start=False, stop=True,
                        ).then_inc(pe_sem, 1)
                        g += 1

        @block.vector
        def _(vector):
            for q in range(NQ):
                vector.wait_ge(pei_sem, q + 1)
                nc.vector.tensor_copy(
                    inter_sb[0:RANK, q * SQ:(q + 1) * SQ], psum_i[0:RANK, :]
                ).then_inc(dve_sem, 1)

        @block.scalar
        def _(scalar):
            for g in range(NGROUP):
                scalar.wait_ge(pe_sem, g + 1)
                if g >= 4:
                    scalar.wait_ge(st_sem, 16 * (g - 3))
                nc.scalar.copy(out_sb[:, g % 4, :], psum_y[:, g % 7, :]).then_inc(
                    ev_sem, 1
                )

        @block.gpsimd
        def _(gpsimd):
            for g in range(NGROUP):
                q, rem = divmod(g, NJ * NT)
                j, t = divmod(rem, NT)
                st = q * NT + t
                gpsimd.wait_ge(ev_sem, g + 1)
                gpsimd.dma_start(
                    y[st * P:(st + 1) * P, j * OJ:(j + 1) * OJ], out_sb[:, g % 4, :]
                ).then_inc(st_sem, 16)

    return nc


_NC_CACHE = {}


def _get_nc():
    if "nc" not in _NC_CACHE:
        _NC_CACHE["nc"] = build_nc()
    return _NC_CACHE["nc"]


def make_in_maps(x, W, bias, lora_a, lora_b, adapter_indices):
    wt = np.ascontiguousarray(W.astype(np.float32).T).astype(ml_dtypes.bfloat16)
    ones = np.ones((1, S), dtype=ml_dtypes.bfloat16)
    in_maps = []
    for c in range(B):
        idx = int(adapter_indices[c])
        xt = np.ascontiguousarray(x[c].astype(np.float32).T).astype(ml_dtypes.bfloat16)
        at = np.ascontiguousarray(lora_a[idx].astype(np.float32).T).astype(
            ml_dtypes.bfloat16)
        bt = np.concatenate(
            [lora_b[idx].astype(np.float32).T, bias.astype(np.float32)[None, :]],
            axis=0).astype(ml_dtypes.bfloat16)
        in_maps.append({"xt": xt, "wt": wt, "at": at, "bt": bt, "ones": ones})
    return in_maps


def kernel(x, W, bias, lora_a, lora_b, adapter_indices):
    nc = _get_nc()
    in_maps = make_in_maps(x, W, bias, lora_a, lora_b, adapter_indices)
    res = run_bass_kernel_spmd(nc, in_maps, list(range(B)))
    out = np.stack([res.results[c]["y"] for c in range(B)], axis=0)
    return out.astype(np.float32)



# revision 2
# speedup vs baseline: 20314.3489x; 20314.3489x over previous
import sys
sys.path.insert(0, '/opt/trn_rl_repo')
import numpy as np
import ml_dtypes

import concourse.bass as bass
import concourse.mybir as mybir
from concourse.bass_utils import run_bass_kernel_spmd

# Problem: y[b,s,o] = x[b]@W.T + bias + (x[b]@a[idx[b]].T)@b[idx[b]].T
# B=8 batch elements -> data-parallel, one per NeuronCore.
B, S, D, RANK = 8, 2048, 4096, 16
P = 128
KT = D // P          # 32 contraction tiles
NQ = 4               # s-quarters
SQ = S // NQ         # 512
NJ = 8               # o-blocks of 512
OJ = D // NJ         # 512
NT = SQ // P         # 4 s-tiles per quarter
NGROUP = NQ * NJ * NT  # 128 output groups of [128 s, 512 o]
NDUMMY = 72          # PE warm-up matmuls covering the initial DMA window

_BF = mybir.dt.bfloat16
_F32 = mybir.dt.float32


def build_nc():
    nc = bass.Bass()
    xt = nc.declare_dram_parameter("xt", [D, S], _BF, isOutput=False)
    wt = nc.declare_dram_parameter("wt", [D, D], _BF, isOutput=False)
    at = nc.declare_dram_parameter("at", [D, RANK], _BF, isOutput=False)
    bt = nc.declare_dram_parameter("bt", [RANK + 1, D], _BF, isOutput=False)
    ones = nc.declare_dram_parameter("ones", [1, S], _BF, isOutput=False)
    y = nc.declare_dram_parameter("y", [S, D], _BF, isOutput=True)

    xt_t = xt.rearrange("(k p) s -> p k s", p=P)
    wt_t = wt.rearrange("(k p) o -> p k o", p=P)
    at_t = at.rearrange("(k p) r -> p k r", p=P)

    with (
        nc.sbuf_tensor([P, 2, KT, SQ], _BF) as x_sb,
        nc.sbuf_tensor([P, 2, KT, OJ], _BF) as w_sb,
        nc.sbuf_tensor([P, KT, RANK], _BF) as at_sb,
        nc.sbuf_tensor([RANK + 1, D], _BF) as bt_sb,
        nc.sbuf_tensor([RANK + 1, S], _BF) as inter_sb,
        nc.sbuf_tensor([P, 4, OJ], _BF) as out_sb,
        nc.psum_tensor([P, 7, OJ], _F32) as psum_y,
        nc.psum_tensor([P, SQ], _F32) as psum_i,
        nc.semaphore("x_sem") as x_sem,
        nc.semaphore("w_sem") as w_sem,
        nc.semaphore("c_sem") as c_sem,
        nc.semaphore("pe_sem") as pe_sem,
        nc.semaphore("pei_sem") as pei_sem,
        nc.semaphore("dve_sem") as dve_sem,
        nc.semaphore("ev_sem") as ev_sem,
        nc.semaphore("st_sem") as st_sem,
        nc.Block() as block,
    ):
        @block.sync
        def _(sync):
            # Order = DMA execution order. Interleave so the tensor engine
            # can start main matmuls as early as possible:
            #   at | x[s0:128] | x[s128:256] | bt | ones | w(j0) | x rest | ...
            sync.dma_start(at_sb[:], at_t).then_inc(c_sem, 16)
            sync.dma_start(
                x_sb[:, 0, :, 0:P], xt_t[:, :, 0:P]
            ).then_inc(x_sem, 16)
            sync.dma_start(
                x_sb[:, 0, :, P:2 * P], xt_t[:, :, P:2 * P]
            ).then_inc(x_sem, 16)
            sync.dma_start(bt_sb[:], bt[:, :]).then_inc(c_sem, 16)
            sync.dma_start(inter_sb[RANK:RANK + 1, :], ones[:, :]).then_inc(c_sem, 16)
            sync.dma_start(
                w_sb[:, 0], wt_t[:, :, 0:OJ]
            ).then_inc(w_sem, 16)
            sync.dma_start(
                x_sb[:, 0, :, 2 * P:3 * P], xt_t[:, :, 2 * P:3 * P]
            ).then_inc(x_sem, 16)
            sync.dma_start(
                x_sb[:, 0, :, 3 * P:SQ], xt_t[:, :, 3 * P:SQ]
            ).then_inc(x_sem, 16)
            for j in range(1, NJ):
                if j >= 2:
                    sync.wait_ge(ev_sem, NT * (j - 1))
                sync.dma_start(
                    w_sb[:, j % 2], wt_t[:, :, j * OJ:(j + 1) * OJ]
                ).then_inc(w_sem, 16)
            for q in range(1, NQ):
                if q >= 2:
                    sync.wait_ge(ev_sem, NJ * NT * (q - 1))
                sync.dma_start(
                    x_sb[:, q % 2], xt_t[:, :, q * SQ:(q + 1) * SQ]
                ).then_inc(x_sem, 16)
                for j in range(NJ):
                    wj = q * NJ + j
                    sync.wait_ge(ev_sem, NT * (wj - 1))
                    sync.dma_start(
                        w_sb[:, j % 2], wt_t[:, :, j * OJ:(j + 1) * OJ]
                    ).then_inc(w_sem, 16)

        @block.tensor
        def _(tensor):
            # Warm-up: keep the PE HAM clock gate open while the first
            # input slabs stream in. Results land in psum_i (overwritten by
            # the first real inter matmul) and are never read.
            for _ in range(NDUMMY):
                nc.tensor.matmul(
                    psum_i[0:RANK, :], at_sb[:, 0, :], at_sb[:],
                    start=True, stop=True,
                )
            tensor.wait_ge(c_sem, 48)
            g = 0
            for q in range(NQ):
                if q == 0:
                    # Special-cased first quarter: main matmuls for the
                    # first two s-tiles run while the rest of x streams in;
                    # inter (needs the full quarter) is deferred.
                    tensor.wait_ge(x_sem, 16)
                    tensor.wait_ge(w_sem, 16)
                    for i in range(KT):
                        nc.tensor.matmul(
                            psum_y[:, 0, :], x_sb[:, 0, i, 0:P], w_sb[:, 0, i, :],
                            start=(i == 0), stop=False,
                        )
                    tensor.wait_ge(x_sem, 32)
                    for i in range(KT):
                        nc.tensor.matmul(
                            psum_y[:, 1, :], x_sb[:, 0, i, P:2 * P], w_sb[:, 0, i, :],
                            start=(i == 0), stop=False,
                        )
                    tensor.wait_ge(x_sem, 64)
                    for i in range(KT):
                        mm = nc.tensor.matmul(
                            psum_i[0:RANK, :], at_sb[:, i, :], x_sb[:, 0, i, :],
                            start=(i == 0), stop=(i == KT - 1),
                        )
                    mm.then_inc(pei_sem, 1)
                    tensor.wait_ge(dve_sem, 1)
                    for t in (0, 1):
                        nc.tensor.matmul(
                            psum_y[:, t, :],
                            inter_sb[:, t * P:(t + 1) * P],
                            bt_sb[:, 0:OJ],
                            start=False, stop=True,
                        ).then_inc(pe_sem, 1)
                        g += 1
                    for t in (2, 3):
                        for i in range(KT):
                            nc.tensor.matmul(
                                psum_y[:, g % 7, :],
                                x_sb[:, 0, i, t * P:(t + 1) * P],
                                w_sb[:, 0, i, :],
                                start=(i == 0), stop=False,
                            )
                        nc.tensor.matmul(
                            psum_y[:, g % 7, :],
                            inter_sb[:, t * P:(t + 1) * P],
                            bt_sb[:, 0:OJ],
                            start=False, stop=True,
                        ).then_inc(pe_sem, 1)
                        g += 1
                    j_start = 1
                else:
                    tensor.wait_ge(x_sem, 64 + 16 * q)
                    tensor.wait_ge(dve_sem, q)     # psum_i WAR
                    for i in range(KT):
                        mm = nc.tensor.matmul(
                            psum_i[0:RANK, :], at_sb[:, i, :], x_sb[:, q % 2, i, :],
                            start=(i == 0), stop=(i == KT - 1),
                        )
                    mm.then_inc(pei_sem, 1)
                    j_start = 0
                for j in range(j_start, NJ):
                    wj = q * NJ + j
                    tensor.wait_ge(w_sem, 16 * (wj + 1))
                    for t in range(NT):
                        st = q * NT + t
                        if g >= 7 and (g - 7) % 2 == 0:
                            tensor.wait_ge(ev_sem, g - 5)
                        for i in range(KT):
                            nc.tensor.matmul(
                                psum_y[:, g % 7, :],
                                x_sb[:, q % 2, i, t * P:(t + 1) * P],
                                w_sb[:, j % 2, i, :],
                                start=(i == 0), stop=False,
                            )
                        if q > 0 and j == j_start and t == 0:
                            tensor.wait_ge(dve_sem, q + 1)
                        nc.tensor.matmul(
                            psum_y[:, g % 7, :],
                            inter_sb[:, st * P:(st + 1) * P],
                            bt_sb[:, j * OJ:(j + 1) * OJ],
                            start=False, stop=True,
                        ).then_inc(pe_sem, 1)
                        g += 1

        @block.vector
        def _(vector):
            for q in range(NQ):
                vector.wait_ge(pei_sem, q + 1)
                nc.vector.tensor_copy(
                    inter_sb[0:RANK, q * SQ:(q + 1) * SQ], psum_i[0:RANK, :]
                ).then_inc(dve_sem, 1)

        @block.scalar
        def _(scalar):
            for g in range(NGROUP):
                scalar.wait_ge(pe_sem, g + 1)
                if g >= 4:
                    scalar.wait_ge(st_sem, 16 * (g - 3))
                nc.scalar.copy(out_sb[:, g % 4, :], psum_y[:, g % 7, :]).then_inc(
                    ev_sem, 1
                )

        @block.gpsimd
        def _(gpsimd):
            for g in range(NGROUP):
                q, rem = divmod(g, NJ * NT)
                j, t = divmod(rem, NT)
                st = q * NT + t
                gpsimd.wait_ge(ev_sem, g + 1)
                gpsimd.dma_start(
                    y[st * P:(st + 1) * P, j * OJ:(j + 1) * OJ], out_sb[:, g % 4, :]
                ).then_inc(st_sem, 16)

    return nc


_NC_CACHE = {}


def _get_nc():
    if "nc" not in _NC_CACHE:
        _NC_CACHE["nc"] = build_nc()
    return _NC_CACHE["nc"]


def make_in_maps(x, W, bias, lora_a, lora_b, adapter_indices):
    wt = np.ascontiguousarray(W.astype(np.float32).T).astype(ml_dtypes.bfloat16)
    ones = np.ones((1, S), dtype=ml_dtypes.bfloat16)
    in_maps = []
    for c in range(B):
        idx = int(adapter_indices[c])
        xt = np.ascontiguousarray(x[c].astype(np.float32).T).astype(ml_dtypes.bfloat16)
        at = np.ascontiguousarray(lora_a[idx].astype(np.float32).T).astype(
            ml_dtypes.bfloat16)
        bt = np.concatenate(
            [lora_b[idx].astype(np.float32).T, bias.astype(np.float32)[None, :]],
            axis=0).astype(ml_dtypes.bfloat16)
        in_maps.append({"xt": xt, "wt": wt, "at": at, "bt": bt, "ones": ones})
    return in_maps


def kernel(x, W, bias, lora_a, lora_b, adapter_indices):
    nc = _get_nc()
    in_maps = make_in_maps(x, W, bias, lora_a, lora_b, adapter_indices)
    res = run_bass_kernel_spmd(nc, in_maps, list(range(B)))
    out = np.stack([res.results[c]["y"] for c in range(B)], axis=0)
    return out.astype(np.float32)


# revision 3
# speedup vs baseline: 20524.4690x; 1.0103x over previous
import sys
sys.path.insert(0, '/opt/trn_rl_repo')
import numpy as np
import ml_dtypes

import concourse.bass as bass
import concourse.mybir as mybir
from concourse.bass_utils import run_bass_kernel_spmd

# Problem: y[b,s,o] = x[b]@W.T + bias + (x[b]@a[idx[b]].T)@b[idx[b]].T
# B=8 batch elements -> data-parallel, one per NeuronCore.
B, S, D, RANK = 8, 2048, 4096, 16
P = 128
KT = D // P          # 32 contraction tiles
NQ = 4               # s-quarters
SQ = S // NQ         # 512
NJ = 8               # o-blocks of 512
OJ = D // NJ         # 512
NT = SQ // P         # 4 s-tiles per quarter
NGROUP = NQ * NJ * NT  # 128 output groups of [128 s, 512 o]
NDUMMY = 64          # PE warm-up matmuls covering the initial DMA window

_BF = mybir.dt.bfloat16
_F32 = mybir.dt.float32


def build_nc():
    nc = bass.Bass()
    # Host-permuted layouts so every DMA moves long contiguous lines per
    # partition: xt[p, (q,t,k,su)], wt[p, (j,k,o)], at[p, (k,r)].
    xt = nc.declare_dram_parameter("xt", [P, NQ * NT * KT * P], _BF, isOutput=False)
    wt = nc.declare_dram_parameter("wt", [P, NJ * KT * OJ], _BF, isOutput=False)
    at = nc.declare_dram_parameter("at", [P, KT * RANK], _BF, isOutput=False)
    bt = nc.declare_dram_parameter("bt", [RANK + 1, D], _BF, isOutput=False)
    ones = nc.declare_dram_parameter("ones", [1, S], _BF, isOutput=False)
    y = nc.declare_dram_parameter("y", [S, D], _BF, isOutput=True)

    xt_r = xt.rearrange("p (q t k su) -> p q t k su", q=NQ, t=NT, k=KT)
    wt_r = wt.rearrange("p (j k o) -> p j k o", j=NJ, k=KT)
    at_r = at.rearrange("p (k r) -> p k r", k=KT)

    with (
        nc.sbuf_tensor([P, 2, NT, KT, P], _BF) as x_sb,
        nc.sbuf_tensor([P, 2, KT, OJ], _BF) as w_sb,
        nc.sbuf_tensor([P, KT, RANK], _BF) as at_sb,
        nc.sbuf_tensor([RANK + 1, D], _BF) as bt_sb,
        nc.sbuf_tensor([RANK + 1, S], _BF) as inter_sb,
        nc.sbuf_tensor([P, 4, OJ], _BF) as out_sb,
        nc.psum_tensor([P, 7, OJ], _F32) as psum_y,
        nc.psum_tensor([P, SQ], _F32) as psum_i,
        nc.semaphore("x_sem") as x_sem,
        nc.semaphore("w_sem") as w_sem,
        nc.semaphore("c_sem") as c_sem,
        nc.semaphore("pe_sem") as pe_sem,
        nc.semaphore("pei_sem") as pei_sem,
        nc.semaphore("dve_sem") as dve_sem,
        nc.semaphore("ev_sem") as ev_sem,
        nc.semaphore("st_sem") as st_sem,
        nc.Block() as block,
    ):
        @block.sync
        def _(sync):
            # Order = DMA execution order. Interleave so the tensor engine
            # can start main matmuls as early as possible.
            sync.dma_start(at_sb[:], at_r).then_inc(c_sem, 16)
            sync.dma_start(x_sb[:, 0, 0], xt_r[:, 0, 0]).then_inc(x_sem, 16)
            sync.dma_start(x_sb[:, 0, 1], xt_r[:, 0, 1]).then_inc(x_sem, 16)
            sync.dma_start(bt_sb[:], bt[:, :]).then_inc(c_sem, 16)
            sync.dma_start(inter_sb[RANK:RANK + 1, :], ones[:, :]).then_inc(c_sem, 16)
            sync.dma_start(w_sb[:, 0], wt_r[:, 0]).then_inc(w_sem, 16)
            sync.dma_start(x_sb[:, 0, 2], xt_r[:, 0, 2]).then_inc(x_sem, 16)
            sync.dma_start(x_sb[:, 0, 3], xt_r[:, 0, 3]).then_inc(x_sem, 16)
            for j in range(1, NJ):
                if j >= 2:
                    sync.wait_ge(ev_sem, NT * (j - 1))
                sync.dma_start(w_sb[:, j % 2], wt_r[:, j]).then_inc(w_sem, 16)
            for q in range(1, NQ):
                if q >= 2:
                    sync.wait_ge(ev_sem, NJ * NT * (q - 1))
                sync.dma_start(x_sb[:, q % 2], xt_r[:, q]).then_inc(x_sem, 16)
                for j in range(NJ):
                    wj = q * NJ + j
                    sync.wait_ge(ev_sem, NT * (wj - 1))
                    sync.dma_start(w_sb[:, j % 2], wt_r[:, j]).then_inc(w_sem, 16)

        @block.tensor
        def _(tensor):
            # Warm-up: keep the PE HAM clock gate open while the first
            # input slabs stream in. Results land in psum_i (overwritten by
            # the first real inter matmul) and are never read.
            for _ in range(NDUMMY):
                nc.tensor.matmul(
                    psum_i[0:RANK, :], at_sb[:, 0, :], at_sb[:],
                    start=True, stop=True,
                )
            tensor.wait_ge(c_sem, 48)
            g = 0
            for q in range(NQ):
                if q == 0:
                    # First quarter: main matmuls for the first two s-tiles
                    # run while the rest of x streams in; inter (needs the
                    # full quarter) is deferred.
                    tensor.wait_ge(x_sem, 16)
                    tensor.wait_ge(w_sem, 16)
                    for i in range(KT):
                        nc.tensor.matmul(
                            psum_y[:, 0, :], x_sb[:, 0, 0, i, :], w_sb[:, 0, i, :],
                            start=(i == 0), stop=False,
                        )
                    tensor.wait_ge(x_sem, 32)
                    for i in range(KT):
                        nc.tensor.matmul(
                            psum_y[:, 1, :], x_sb[:, 0, 1, i, :], w_sb[:, 0, i, :],
                            start=(i == 0), stop=False,
                        )
                    tensor.wait_ge(x_sem, 64)
                    for i in range(KT):
                        mm = nc.tensor.matmul(
                            psum_i[0:RANK, :], at_sb[:, i, :], x_sb[:, 0, :, i, :],
                            start=(i == 0), stop=(i == KT - 1),
                        )
                    mm.then_inc(pei_sem, 1)
                    tensor.wait_ge(dve_sem, 1)
                    for t in (0, 1):
                        nc.tensor.matmul(
                            psum_y[:, t, :],
                            inter_sb[:, t * P:(t + 1) * P],
                            bt_sb[:, 0:OJ],
                            start=False, stop=True,
                        ).then_inc(pe_sem, 1)
                        g += 1
                    for t in (2, 3):
                        for i in range(KT):
                            nc.tensor.matmul(
                                psum_y[:, g % 7, :],
                                x_sb[:, 0, t, i, :],
                                w_sb[:, 0, i, :],
                                start=(i == 0), stop=False,
                            )
                        nc.tensor.matmul(
                            psum_y[:, g % 7, :],
                            inter_sb[:, t * P:(t + 1) * P],
                            bt_sb[:, 0:OJ],
                            start=False, stop=True,
                        ).then_inc(pe_sem, 1)
                        g += 1
                    j_start = 1
                else:
                    tensor.wait_ge(x_sem, 64 + 16 * q)
                    tensor.wait_ge(dve_sem, q)     # psum_i WAR
                    for i in range(KT):
                        mm = nc.tensor.matmul(
                            psum_i[0:RANK, :], at_sb[:, i, :], x_sb[:, q % 2, :, i, :],
                            start=(i == 0), stop=(i == KT - 1),
                        )
                    mm.then_inc(pei_sem, 1)
                    j_start = 0
                for j in range(j_start, NJ):
                    wj = q * NJ + j
                    tensor.wait_ge(w_sem, 16 * (wj + 1))
                    for t in range(NT):
                        st = q * NT + t
                        if g >= 7 and (g - 7) % 4 == 0:
                            tensor.wait_ge(ev_sem, g - 3)
                        for i in range(KT):
                            nc.tensor.matmul(
                                psum_y[:, g % 7, :],
                                x_sb[:, q % 2, t, i, :],
                                w_sb[:, j % 2, i, :],
                                start=(i == 0), stop=False,
                            )
                        if q > 0 and j == j_start and t == 0:
                            tensor.wait_ge(dve_sem, q + 1)
                        nc.tensor.matmul(
                            psum_y[:, g % 7, :],
                            inter_sb[:, st * P:(st + 1) * P],
                            bt_sb[:, j * OJ:(j + 1) * OJ],
                            start=False, stop=True,
                        ).then_inc(pe_sem, 1)
                        g += 1

        @block.vector
        def _(vector):
            for q in range(NQ):
                vector.wait_ge(pei_sem, q + 1)
                nc.vector.tensor_copy(
                    inter_sb[0:RANK, q * SQ:(q + 1) * SQ], psum_i[0:RANK, :]
                ).then_inc(dve_sem, 1)

        @block.scalar
        def _(scalar):
            for g in range(NGROUP):
                scalar.wait_ge(pe_sem, g + 1)
                if g >= 4:
                    scalar.wait_ge(st_sem, 16 * (g - 3))
                nc.scalar.copy(out_sb[:, g % 4, :], psum_y[:, g % 7, :]).then_inc(
                    ev_sem, 1
                )

        @block.gpsimd
        def _(gpsimd):
            for g in range(NGROUP):
                q, rem = divmod(g, NJ * NT)
                j, t = divmod(rem, NT)
                st = q * NT + t
                gpsimd.wait_ge(ev_sem, g + 1)
                gpsimd.dma_start(
                    y[st * P:(st + 1) * P, j * OJ:(j + 1) * OJ], out_sb[:, g % 4, :]
                ).then_inc(st_sem, 16)

    return nc


_NC_CACHE = {}


def _get_nc():
    if "nc" not in _NC_CACHE:
        _NC_CACHE["nc"] = build_nc()
    return _NC_CACHE["nc"]


def make_in_maps(x, W, bias, lora_a, lora_b, adapter_indices):
    # W.T laid out [p, (j,k,o)] so each w j-slab is one contiguous 32KB/line DMA.
    wtT = W.astype(np.float32).T                       # [D_in, D_out]
    wt = np.ascontiguousarray(
        wtT.reshape(KT, P, NJ, OJ).transpose(1, 2, 0, 3).reshape(P, NJ * KT * OJ)
    ).astype(ml_dtypes.bfloat16)
    ones = np.ones((1, S), dtype=ml_dtypes.bfloat16)
    in_maps = []
    for c in range(B):
        idx = int(adapter_indices[c])
        # x[c].T laid out [p, (q,t,k,su)] -> 8KB contiguous lines per s-tile.
        xtT = x[c].astype(np.float32).T                # [D_in, S]
        xt = np.ascontiguousarray(
            xtT.reshape(KT, P, NQ, NT, P).transpose(1, 2, 3, 0, 4)
            .reshape(P, NQ * NT * KT * P)
        ).astype(ml_dtypes.bfloat16)
        # lora_a[idx].T laid out [p, (k,r)].
        atT = lora_a[idx].astype(np.float32).T         # [D_in, RANK]
        at = np.ascontiguousarray(
            atT.reshape(KT, P, RANK).transpose(1, 0, 2).reshape(P, KT * RANK)
        ).astype(ml_dtypes.bfloat16)
        bt = np.concatenate(
            [lora_b[idx].astype(np.float32).T, bias.astype(np.float32)[None, :]],
            axis=0).astype(ml_dtypes.bfloat16)
        in_maps.append({"xt": xt, "wt": wt, "at": at, "bt": bt, "ones": ones})
    return in_maps


def kernel(x, W, bias, lora_a, lora_b, adapter_indices):
    nc = _get_nc()
    in_maps = make_in_maps(x, W, bias, lora_a, lora_b, adapter_indices)
    res = run_bass_kernel_spmd(nc, in_maps, list(range(B)))
    out = np.stack([res.results[c]["y"] for c in range(B)], axis=0)
    return out.astype(np.float32)


# revision 16
# speedup vs baseline: 21251.9567x; 1.0354x over previous
import sys
sys.path.insert(0, '/opt/trn_rl_repo')
import numpy as np
import ml_dtypes

import concourse.bass as bass
import concourse.mybir as mybir
from concourse.bass_utils import run_bass_kernel_spmd

# Problem: y[b,s,o] = x[b]@W.T + bias + (x[b]@a[idx[b]].T)@b[idx[b]].T
# B=8 batch elements -> data-parallel, one per NeuronCore.
B, S, D, RANK = 8, 2048, 4096, 16
P = 128
KT = D // P          # 32 contraction tiles
NQ = 4               # s-quarters
SQ = S // NQ         # 512
NJ = 8               # o-blocks of 512
OJ = D // NJ         # 512
NT = SQ // P         # 4 s-tiles per quarter
NGROUP = NQ * NJ * NT  # 128 output groups of [128 s, 512 o]
NDUMMY = 24          # PE warm-up matmuls covering the initial DMA window
WCH = KT // 4        # first w slab arrives in 4 k-chunks of 8 tiles

_BF = mybir.dt.bfloat16
_F32 = mybir.dt.float32


def build_nc():
    nc = bass.Bass()
    # Host-permuted layouts so every DMA moves long contiguous lines per
    # partition: xt[p, (q,t,k,su)], wt[p, (j,k,o)], at[p, (k,r)].
    xt = nc.declare_dram_parameter("xt", [P, NQ * NT * KT * P], _BF, isOutput=False)
    wt = nc.declare_dram_parameter("wt", [P, NJ * KT * OJ], _BF, isOutput=False)
    at = nc.declare_dram_parameter("at", [P, KT * RANK], _BF, isOutput=False)
    # bt is zero-padded to the full 128 contraction rows (rows 0:16 = b.T,
    # row 32 = bias, rest zero) so the b-part matmul's stationary operand is
    # a full 128-row tile: NumWeights==128 keeps FWL enabled and the
    # weight-load pipelined like every main matmul.
    bt = nc.declare_dram_parameter("bt", [P, D], _BF, isOutput=False)
    ones = nc.declare_dram_parameter("ones", [1, S], _BF, isOutput=False)
    y = nc.declare_dram_parameter("y", [S, D], _BF, isOutput=True)

    xt_r = xt.rearrange("p (q t k su) -> p q t k su", q=NQ, t=NT, k=KT)
    wt_r = wt.rearrange("p (j k o) -> p j k o", j=NJ, k=KT)
    at_r = at.rearrange("p (k r) -> p k r", k=KT)

    with (
        nc.sbuf_tensor([P, 2, NT, KT, P], _BF) as x_sb,
        nc.sbuf_tensor([P, 2, KT, OJ], _BF) as w_sb,
        nc.sbuf_tensor([P, KT, RANK], _BF) as at_sb,
        nc.sbuf_tensor([P, D], _BF) as bt_sb,
        nc.sbuf_tensor([P, S], _BF) as inter_sb,
        nc.sbuf_tensor([P, 4, OJ], _BF) as out_sb,
        nc.psum_tensor([P, 7, OJ], _F32) as psum_y,
        nc.psum_tensor([P, SQ], _F32) as psum_i,
        nc.semaphore("x_sem") as x_sem,
        nc.semaphore("w_sem") as w_sem,
        nc.semaphore("pe_sem") as pe_sem,
        nc.semaphore("pei_sem") as pei_sem,
        nc.semaphore("dve_sem") as dve_sem,
        nc.semaphore("ev_sem") as ev_sem,
        nc.semaphore("st_sem") as st_sem,
        nc.Block() as block,
    ):
        @block.sync
        def _(sync):
            # Order = DMA execution order. Interleave so the tensor engine
            # can start main matmuls as early as possible: small tables
            # first, then the first x s-tile, then the first w slab in
            # k-chunks the k-loop can consume as they land.
            sync.dma_start(at_sb[:], at_r).then_inc(x_sem, 16)
            sync.dma_start(x_sb[:, 0, 0], xt_r[:, 0, 0]).then_inc(x_sem, 16)
            for kc in range(4):
                sync.dma_start(
                    w_sb[:, 0, kc * WCH:(kc + 1) * WCH],
                    wt_r[:, 0, kc * WCH:(kc + 1) * WCH],
                ).then_inc(w_sem, 16)
            sync.dma_start(bt_sb[:], bt[:, :]).then_inc(x_sem, 16)
            # inter_sb is zeroed by the vector engine first (garbage rows
            # would poison the 128-row contraction: NaN*0 = NaN); only then
            # write the ones row used for the bias contribution.
            sync.wait_ge(dve_sem, 1)
            sync.dma_start(inter_sb[32:33, :], ones[:, :]).then_inc(x_sem, 16)
            sync.dma_start(x_sb[:, 0, 1], xt_r[:, 0, 1]).then_inc(x_sem, 16)
            sync.dma_start(x_sb[:, 0, 2], xt_r[:, 0, 2]).then_inc(x_sem, 16)
            sync.dma_start(x_sb[:, 0, 3], xt_r[:, 0, 3]).then_inc(x_sem, 16)
            for j in range(1, NJ):
                if j >= 2:
                    sync.wait_ge(ev_sem, NT * (j - 1))
                sync.dma_start(w_sb[:, j % 2], wt_r[:, j]).then_inc(w_sem, 16)
            for q in range(1, NQ):
                if q >= 2:
                    sync.wait_ge(ev_sem, NJ * NT * (q - 1))
                sync.dma_start(x_sb[:, q % 2], xt_r[:, q]).then_inc(x_sem, 16)
                for j in range(NJ):
                    wj = q * NJ + j
                    sync.wait_ge(ev_sem, NT * (wj - 1))
                    sync.dma_start(w_sb[:, j % 2], wt_r[:, j]).then_inc(w_sem, 16)

        @block.tensor
        def _(tensor):
            # Warm-up: keep the PE HAM clock gate open while the first
            # input slabs stream in. Results land in psum_i (overwritten by
            # the first real inter matmul) and are never read.
            for _ in range(NDUMMY):
                nc.tensor.matmul(
                    psum_i[0:RANK, :], at_sb[:, 0, :], at_sb[:],
                    start=True, stop=True,
                )
            g = 0
            for q in range(NQ):
                if q == 0:
                    # First quarter: the first s-tile's k-loop consumes w
                    # k-chunks as they land; the remaining s-tiles run while
                    # the rest of x streams in; inter (needs the full
                    # quarter) is deferred.
                    tensor.wait_ge(x_sem, 32)
                    for i in range(KT):
                        if i % WCH == 0:
                            tensor.wait_ge(w_sem, 16 * (i // WCH + 1))
                        nc.tensor.matmul(
                            psum_y[:, 0, :], x_sb[:, 0, 0, i, :], w_sb[:, 0, i, :],
                            start=(i == 0), stop=False,
                        )
                    tensor.wait_ge(x_sem, 80)
                    for i in range(KT):
                        nc.tensor.matmul(
                            psum_y[:, 1, :], x_sb[:, 0, 1, i, :], w_sb[:, 0, i, :],
                            start=(i == 0), stop=False,
                        )
                    tensor.wait_ge(x_sem, 112)
                    for i in range(KT):
                        mm = nc.tensor.matmul(
                            psum_i[0:RANK, :], at_sb[:, i, :], x_sb[:, 0, :, i, :],
                            start=(i == 0), stop=(i == KT - 1),
                        )
                    mm.then_inc(pei_sem, 1)
                    tensor.wait_ge(dve_sem, 2)
                    for t in (0, 1):
                        nc.tensor.matmul(
                            psum_y[:, t, :],
                            inter_sb[:, t * P:(t + 1) * P],
                            bt_sb[:, 0:OJ],
                            start=False, stop=True,
                        ).then_inc(pe_sem, 1)
                        g += 1
                    for t in (2, 3):
                        for i in range(KT):
                            nc.tensor.matmul(
                                psum_y[:, g % 7, :],
                                x_sb[:, 0, t, i, :],
                                w_sb[:, 0, i, :],
                                start=(i == 0), stop=False,
                            )
                        nc.tensor.matmul(
                            psum_y[:, g % 7, :],
                            inter_sb[:, t * P:(t + 1) * P],
                            bt_sb[:, 0:OJ],
                            start=False, stop=True,
                        ).then_inc(pe_sem, 1)
                        g += 1
                    j_start = 1
                else:
                    tensor.wait_ge(x_sem, 112 + 16 * q)
                    tensor.wait_ge(dve_sem, q + 1)  # psum_i WAR
                    for i in range(KT):
                        mm = nc.tensor.matmul(
                            psum_i[0:RANK, :], at_sb[:, i, :], x_sb[:, q % 2, :, i, :],
                            start=(i == 0), stop=(i == KT - 1),
                        )
                    mm.then_inc(pei_sem, 1)
                    j_start = 0
                for j in range(j_start, NJ):
                    wj = q * NJ + j
                    tensor.wait_ge(w_sem, 48 + 16 * (wj + 1))
                    for t in range(NT):
                        st = q * NT + t
                        if t == 0 and g >= 4:
                            tensor.wait_ge(ev_sem, g - 3)
                        for i in range(KT):
                            nc.tensor.matmul(
                                psum_y[:, g % 7, :],
                                x_sb[:, q % 2, t, i, :],
                                w_sb[:, j % 2, i, :],
                                start=(i == 0), stop=False,
                            )
                        if q > 0 and j == j_start and t == 0:
                            tensor.wait_ge(dve_sem, q + 2)
                        nc.tensor.matmul(
                            psum_y[:, g % 7, :],
                            inter_sb[:, st * P:(st + 1) * P],
                            bt_sb[:, j * OJ:(j + 1) * OJ],
                            start=False, stop=True,
                        ).then_inc(pe_sem, 1)
                        g += 1

        @block.vector
        def _(vector):
            nc.vector.memset(inter_sb[:], 0.0).then_inc(dve_sem, 1)
            for q in range(NQ):
                vector.wait_ge(pei_sem, q + 1)
                nc.vector.tensor_copy(
                    inter_sb[0:RANK, q * SQ:(q + 1) * SQ], psum_i[0:RANK, :]
                ).then_inc(dve_sem, 1)

        @block.scalar
        def _(scalar):
            for g in range(NGROUP):
                scalar.wait_ge(pe_sem, g + 1)
                if g >= 4:
                    scalar.wait_ge(st_sem, 16 * (g - 3))
                nc.scalar.copy(out_sb[:, g % 4, :], psum_y[:, g % 7, :]).then_inc(
                    ev_sem, 1
                )

        @block.gpsimd
        def _(gpsimd):
            for g in range(NGROUP):
                q, rem = divmod(g, NJ * NT)
                j, t = divmod(rem, NT)
                st = q * NT + t
                gpsimd.wait_ge(ev_sem, g + 1)
                gpsimd.dma_start(
                    y[st * P:(st + 1) * P, j * OJ:(j + 1) * OJ], out_sb[:, g % 4, :]
                ).then_inc(st_sem, 16)

    return nc


_NC_CACHE = {}


def _get_nc():
    if "nc" not in _NC_CACHE:
        _NC_CACHE["nc"] = build_nc()
    return _NC_CACHE["nc"]


def make_in_maps(x, W, bias, lora_a, lora_b, adapter_indices):
    # W.T laid out [p, (j,k,o)] so each w j-slab is one contiguous 32KB/line DMA.
    wtT = W.astype(np.float32).T                       # [D_in, D_out]
    wt = np.ascontiguousarray(
        wtT.reshape(KT, P, NJ, OJ).transpose(1, 2, 0, 3).reshape(P, NJ * KT * OJ)
    ).astype(ml_dtypes.bfloat16)
    ones = np.ones((1, S), dtype=ml_dtypes.bfloat16)
    in_maps = []
    for c in range(B):
        idx = int(adapter_indices[c])
        # x[c].T laid out [p, (q,t,k,su)] -> 8KB contiguous lines per s-tile.
        xtT = x[c].astype(np.float32).T                # [D_in, S]
        xt = np.ascontiguousarray(
            xtT.reshape(KT, P, NQ, NT, P).transpose(1, 2, 3, 0, 4)
            .reshape(P, NQ * NT * KT * P)
        ).astype(ml_dtypes.bfloat16)
        # lora_a[idx].T laid out [p, (k,r)].
        atT = lora_a[idx].astype(np.float32).T         # [D_in, RANK]
        at = np.ascontiguousarray(
            atT.reshape(KT, P, RANK).transpose(1, 0, 2).reshape(P, KT * RANK)
        ).astype(ml_dtypes.bfloat16)
        btp = np.zeros((P, D), dtype=np.float32)
        btp[0:RANK] = lora_b[idx].astype(np.float32).T
        btp[32] = bias.astype(np.float32)
        bt = btp.astype(ml_dtypes.bfloat16)
        in_maps.append({"xt": xt, "wt": wt, "at": at, "bt": bt, "ones": ones})
    return in_maps


def kernel(x, W, bias, lora_a, lora_b, adapter_indices):
    nc = _get_nc()
    in_maps = make_in_maps(x, W, bias, lora_a, lora_b, adapter_indices)
    res = run_bass_kernel_spmd(nc, in_maps, list(range(B)))
    out = np.stack([res.results[c]["y"] for c in range(B)], axis=0)
    return out.astype(np.float32)
